# revision 40
# baseline (speedup 1.0000x reference)
"""Trainium2 Bass kernel for nn_Attention_62130996904205.

Full computation (reference):
    q = left @ Wq;  k,v = split(right @ Wkv)
    per head: S = scale * q k^T; S = where(mask, S, -1e7)
    out = (softmax(S) @ v) rearranged @ Wout + bout

Sharding: 8 cores = (batch b in 0..3) x (head-half in 0..1).  Host sums
the two head-half partials per batch and adds bout.

On-chip layout ("S^T scheme"): kv token index n stays on the partition
axis.  Per 128-token tile nt, BOTH heads of the pair live in one PSUM
tile s_nt[128, 2, 512] -- the two 64-contraction score matmuls (row
tiles (0,0) and (64,0)) share one buffer-release event so they issue
back-to-back and stream concurrently through the PE array.

exp+mask per nt uses one int16 mask tile (broadcast over the head dim):
  - Schraudolph nts: DVE fused  pm_bits = int16(A*s + mb),
        mb = mask ? 16249 : 5120   (A = 128/ln2 folded into Wq)
    B=16249 keeps the fast-exp mean-calibrated (+-3% sawtooth, no
    systematic bias vs the true-exp nts).  Masked entries ~1e-26.
  - true-exp nts: scalar-engine Exp then a DVE mask multiply with
        mb = mask ? 16256 : 5120   bitcast to bf16 = {1.0, 6.5e-27}.
O^T accumulates via [v | 1]-augmented matmuls, so softmax denominators
ride along free.
"""

import numpy as np
import ml_dtypes

import concourse.bass as bass
import concourse.mybir as mybir
import concourse.tile as tile
from concourse import bacc
from concourse.bass_utils import run_bass_kernel_spmd

BF16 = ml_dtypes.bfloat16

SCHR_A = 128.0 / np.log(2.0)          # 184.6627
SCHR_B = 16249                        # calibrated fast-exp bias
MB_ONE = 16256                        # bf16 bits of 1.0
MB_ZERO = 5120                        # positive-tiny for both paths

# nts (mod 16) handled by the DVE Schraudolph path; rest use ACT Exp.
DVE16 = frozenset({0, 2, 5, 7, 11, 14, 15})

TRACE = False
LAST_RESULTS = None


def build_core(M=1024, N=4096, DQ=512, H=4, DH=64):
    dt = mybir.dt
    f32, bf16, i16 = dt.float32, dt.bfloat16, dt.int16
    D = H * DH
    KT = DQ // 128
    NT = N // 128
    MCH = min(512, M)
    NMC = M // MCH
    SW = 2 * MCH
    DA = DH + 1
    KT2 = D // 128
    VW = H * DH

    assert M % MCH == 0 and N % 256 == 0 and DQ % 128 == 0 and D % 128 == 0

    nc = bacc.Bacc("TRN2", target_bir_lowering=False, debug=False)

    leftT = nc.dram_tensor("leftT", [DQ, M], bf16, kind="ExternalInput")
    rightT = nc.dram_tensor("rightT", [DQ, N], bf16, kind="ExternalInput")
    NG = NT // 4
    maskb = nc.dram_tensor("maskb", [NMC, NG, 128, 4 * MCH], i16, kind="ExternalInput")
    wq = nc.dram_tensor("wq", [DQ, D], bf16, kind="ExternalInput")
    wk = nc.dram_tensor("wk", [DQ, D], bf16, kind="ExternalInput")
    wv = nc.dram_tensor("wv", [DQ, D], bf16, kind="ExternalInput")
    wout = nc.dram_tensor("wout", [D, DQ], bf16, kind="ExternalInput")
    out_p = nc.dram_tensor("out_p", [M, DQ], f32, kind="ExternalOutput")

    EXP = mybir.ActivationFunctionType.Exp
    MUL = mybir.AluOpType.mult
    ADD = mybir.AluOpType.add

    with tile.TileContext(nc) as tc:
        with (
            tc.tile_pool(name="sing", bufs=1) as sing,
            tc.tile_pool(name="spool", bufs=3, space="PSUM") as spool,
            tc.tile_pool(name="opool", bufs=1, space="PSUM") as opool,
            tc.tile_pool(name="mpool", bufs=11) as mpool,
            tc.tile_pool(name="ppool", bufs=11) as ppool,
            tc.tile_pool(name="smallp", bufs=2) as smallp,
            tc.tile_pool(name="outp", bufs=5) as outp,
        ):
            # ---- bulk loads, all on the sync queue in dependency order ----
            wq_sb = sing.tile([128, KT, D], bf16, tag="wq")
            nc.sync.dma_start(out=wq_sb, in_=wq.rearrange("(kt p) d -> p kt d", p=128))
            leftT_sb = []
            for kt in range(KT):
                t = sing.tile([128, M], bf16, tag=f"leftT{kt}", name=f"leftT{kt}")
                nc.sync.dma_start(out=t, in_=leftT[kt * 128 : (kt + 1) * 128, :])
                leftT_sb.append(t)
            wk_sb = sing.tile([128, KT, D], bf16, tag="wk")
            nc.sync.dma_start(out=wk_sb, in_=wk.rearrange("(kt p) d -> p kt d", p=128))
            RCH = min(N, 1024)
            _EARLY_MASKS = True
            rightT_sb = [
                sing.tile([128, N], bf16, tag=f"rightT{kt}", name=f"rightT{kt}")
                for kt in range(KT)
            ]
            _early_msk_loads = []  # filled right below once load_msk_group exists
            wv_sb = sing.tile([128, KT, D], bf16, tag="wv")
            nc.sync.dma_start(out=wv_sb, in_=wv.rearrange("(kt p) d -> p kt d", p=128))

            qT2 = [sing.tile([128, M], bf16, tag=f"qT{h}", name=f"qT{h}") for h in range(H // 2)]
            kT2 = [sing.tile([128, N], bf16, tag=f"kT{h}", name=f"kT{h}") for h in range(H // 2)]
            u_sb = [sing.tile([128, M], bf16, tag=f"u{p}", name=f"u{p}") for p in range(KT2)]
            v_aug = sing.tile([128, NT, H, DA], bf16, tag="vaug")
            nc.vector.memset(v_aug[:, :, :, DH : DH + 1], 1.0)

            # masks: one [128, 4, MCH] int16 tile per (mc, 4-nt group), on the
            # sync queue (batched: 16 DMA issues total, not 64)
            msks = {}

            def load_msk_group(mc, g):
                mg = mpool.tile(
                    [128, 4, 1, MCH], i16, tag="msk", name=f"msk{mc}_{g}"
                )
                nc.sync.dma_start(out=mg, in_=maskb[mc, g])
                for j in range(4):
                    msks[(mc, 4 * g + j)] = (mg, j)

            # interleave remaining bulk with mc0 mask groups in deadline order
            load_msk_group(0, 0)
            load_msk_group(0, 1)
            for kt in range(KT):   # first column chunk: needed by k_chunk(0,0)
                nc.sync.dma_start(
                    out=rightT_sb[kt][:, 0:RCH],
                    in_=rightT[kt * 128 : (kt + 1) * 128, 0:RCH],
                )
            for c in range(1, N // RCH):
                for kt in range(KT):
                    nc.sync.dma_start(
                        out=rightT_sb[kt][:, c * RCH : (c + 1) * RCH],
                        in_=rightT[
                            kt * 128 : (kt + 1) * 128, c * RCH : (c + 1) * RCH
                        ],
                    )
                load_msk_group(0, 1 + c)
            for g in range(N // RCH + 1, NG):
                load_msk_group(0, g)
            wout_sb = sing.tile([128, KT2, DQ], bf16, tag="wout")
            nc.sync.dma_start(
                out=wout_sb, in_=wout.rearrange("(kt p) d -> p kt d", p=128)
            )

            # ---- q projection ----
            for t2 in range(H // 2):
                ps = spool.tile([128, 2, MCH], f32, tag="s")
                for mh in range(M // MCH):
                    for kt in range(KT):
                        nc.tensor.matmul(
                            ps[:, mh, :],
                            lhsT=wq_sb[:, kt, t2 * 128 : (t2 + 1) * 128],
                            rhs=leftT_sb[kt][:, mh * MCH : (mh + 1) * MCH],
                            start=(kt == 0),
                            stop=(kt == KT - 1),
                        )
                nc.scalar.copy(out=qT2[t2][:, :], in_=ps[:, :, :])

            CW = min(SW, N)
            NKC = N // CW

            def k_chunk(t2, cp):
                ps = spool.tile([128, 2, MCH], f32, tag="s", name="kps")
                for half in range(CW // MCH):
                    for kt in range(KT):
                        nc.tensor.matmul(
                            ps[:, half, :],
                            lhsT=wk_sb[:, kt, t2 * 128 : (t2 + 1) * 128],
                            rhs=rightT_sb[kt][
                                :, cp * CW + half * MCH : cp * CW + (half + 1) * MCH
                            ],
                            start=(kt == 0),
                            stop=(kt == KT - 1),
                        )
                nc.scalar.copy(
                    out=kT2[t2][:, cp * CW : (cp + 1) * CW], in_=ps[:, :, :]
                )

            def v_nt(nt):
                ps = spool.tile([128, 2, MCH], f32, tag="s", name="vps")
                for kt in range(KT):
                    nc.tensor.matmul(
                        ps[:, 0, 0:VW],
                        lhsT=rightT_sb[kt][:, nt * 128 : (nt + 1) * 128],
                        rhs=wv_sb[:, kt, :],
                        start=(kt == 0),
                        stop=(kt == KT - 1),
                    )
                nc.scalar.copy(out=v_aug[:, nt, :, 0:DH], in_=ps[:, 0, 0:VW])

            def outproj_mt(mt):
                ps = spool.tile([128, 2, MCH], f32, tag="s", name="ops")
                for p2 in range(KT2):
                    nc.tensor.matmul(
                        ps[:, 0, 0:DQ],
                        lhsT=u_sb[p2][:, mt * 128 : (mt + 1) * 128],
                        rhs=wout_sb[:, p2, :],
                        start=(p2 == 0),
                        stop=(p2 == KT2 - 1),
                    )
                ob = outp.tile([128, DQ], f32, tag="ob")
                if mt % 2 == 0:
                    nc.scalar.copy(out=ob, in_=ps[:, 0, 0:DQ])
                else:
                    nc.vector.tensor_copy(ob, ps[:, 0, 0:DQ])
                nc.sync.dma_start(out=out_p[mt * 128 : (mt + 1) * 128, :], in_=ob)

            # ---- upfront projection work (overlaps the initial DMA wave) ----
            UPFRONT_V = min(NT, 6)
            k_chunk(0, 0)
            for nt in range(UPFRONT_V):
                v_nt(nt)

            # ---- per-phase filler: (deadline_nt_slot, fn) sorted ----
            def phase_filler(mc, hp):
                items = []
                if mc == 0 and hp == 0:
                    for cp in range(1, NKC):
                        items.append((max(0, 8 * cp - 6), lambda cp=cp: k_chunk(0, cp)))
                    for nt in range(UPFRONT_V, NT):
                        items.append((max(0, nt - 5), lambda nt=nt: v_nt(nt)))
                    if H > 2:
                        # k(1,0) must be emitted before phase (0,1) reads it
                        items.append((NT - 1, lambda: k_chunk(1, 0)))
                elif mc == 0 and hp == 1 and H > 2:
                    for cp in range(1, NKC):
                        items.append((max(0, 8 * cp - 6), lambda cp=cp: k_chunk(1, cp)))
                elif mc == 1 and hp == 0:
                    for j in range(MCH // 128):
                        items.append((4 + 7 * j, lambda j=j: outproj_mt(j)))
                items.sort(key=lambda x: x[0])
                return items

            DEPTH = 5
            for mc in range(NMC):
                for hp in range(H // 2):
                    filler = phase_filler(mc, hp)
                    o_ps = [
                        opool.tile([DA, MCH], f32, tag=f"o{i}", name=f"o{i}")
                        for i in range(2)
                    ]
                    oq = []
                    started = [False, False]

                    def make_flush(o_ps, oq, started, hp):
                        def flush_one():
                            pm, nt_ = oq.pop(0)
                            for i in range(2):
                                nc.tensor.matmul(
                                    o_ps[i],
                                    lhsT=v_aug[:, nt_, 2 * hp + i, :],
                                    rhs=pm[:, i, :],
                                    start=(not started[i]),
                                    stop=(nt_ == NT - 1),
                                )
                                started[i] = True
                        return flush_one

                    flush_one = make_flush(o_ps, oq, started, hp)

                    for nt in range(NT):
                        mg, mj = msks[(mc, nt)]
                        # rolling prefetch of next-mc masks during hp=1
                        if hp == 1 and mc + 1 < NMC and nt % 4 == 0:
                            load_msk_group(mc + 1, nt // 4)
                        s_nt = spool.tile([128, 2, MCH], f32, tag="s", name=f"s{nt}")
                        for i in range(2):
                            lo = 64 * i
                            nc.tensor.matmul(
                                s_nt[:, i, :],
                                lhsT=kT2[hp][lo : lo + 64, nt * 128 : (nt + 1) * 128],
                                rhs=qT2[hp][lo : lo + 64, mc * MCH : (mc + 1) * MCH],
                                start=True,
                                stop=True,
                                tile_position=(lo, 0),
                            )
                        pm = ppool.tile([128, 2, MCH], bf16, tag="p")
                        if nt % 16 in DVE16:
                            nc.vector.scalar_tensor_tensor(
                                out=pm.bitcast(i16),
                                in0=s_nt[:, :, :],
                                scalar=1.0,
                                in1=mg[:, mj].to_broadcast((128, 2, MCH)),
                                op0=MUL,
                                op1=ADD,
                            )
                        else:
                            p_sb = ppool.tile([128, 2, MCH], bf16, tag="p")
                            nc.scalar.activation(
                                p_sb, s_nt, EXP, scale=float(1.0 / SCHR_A)
                            )
                            nc.vector.tensor_mul(
                                pm,
                                p_sb,
                                mg.bitcast(bf16)[:, mj].to_broadcast(
                                    (128, 2, MCH)
                                ),
                            )
                        oq.append((pm, nt))
                        if len(oq) > (DEPTH if nt < NT - 4 else 2):
                            flush_one()
                        while filler and filler[0][0] <= nt:
                            filler.pop(0)[1]()
                    while filler:
                        filler.pop(0)[1]()
                    while oq:
                        flush_one()
                    for i in range(2):
                        h = 2 * hp + i
                        rdc = smallp.tile([1, MCH], f32, tag="rdc", name=f"rdc{i}")
                        nc.scalar.copy(out=rdc, in_=o_ps[i][DH : DH + 1, :])
                        rd = smallp.tile([1, MCH], f32, tag="rd", name=f"rd{i}")
                        nc.vector.reciprocal_approx_fast(out=rd, in_=rdc)
                        bd = smallp.tile([64, MCH], f32, tag="bd", name=f"bd{i}")
                        nc.gpsimd.partition_broadcast(bd, rd)
                        nc.vector.tensor_mul(
                            u_sb[h // 2][
                                (h % 2) * 64 : (h % 2) * 64 + 64,
                                mc * MCH : (mc + 1) * MCH,
                            ],
                            o_ps[i][0:DH, :],
                            bd,
                        )
            # tail: last mc's output projections
            for mt in range((NMC - 1) * MCH // 128, NMC * MCH // 128):
                outproj_mt(mt)

    nc.finalize()
    return nc


_NC_CACHE = {}


def _get_nc(key=(1024, 4096, 512, 4, 64)):
    if key not in _NC_CACHE:
        _NC_CACHE[key] = build_core(*key)
    return _NC_CACHE[key]


def kernel(left, right, mask, Wq, Wkv, Wout, bout):
    """Full-input entry point: shards across 8 neuron cores, returns the
    full (B, M, DQ) output."""
    global LAST_RESULTS
    B, M, DQmat = left.shape
    _, N, DC = right.shape
    H, DH = 8, 64
    D = H * DH
    Hc = H // 2
    scale = DH ** -0.5
    NMC = M // 512
    NT = N // 128

    left = np.asarray(left, dtype=np.float32)
    right = np.asarray(right, dtype=np.float32)
    Wq = np.asarray(Wq, dtype=np.float32)
    Wkv = np.asarray(Wkv, dtype=np.float32)
    Wout = np.asarray(Wout, dtype=np.float32)
    bout = np.asarray(bout, dtype=np.float32)

    # 1/sqrt(DH) and the Schraudolph log2-scale are folded into Wq.
    # Wk,Wv are scaled x16 for fp8 e4m3 resolution; compensated by Wq/16
    # and Wout/16 respectively (the v scaling cancels in the softmax
    # denominator only for the value rows, so Wout absorbs it).
    Wqs = (Wq * (scale * SCHR_A)).astype(BF16)
    Wk = Wkv[:, :D].astype(BF16)
    Wv = Wkv[:, D:].astype(BF16)
    WoutB = Wout.astype(BF16)

    leftT = np.ascontiguousarray(left.transpose(0, 2, 1)).astype(BF16)
    rightT = np.ascontiguousarray(right.transpose(0, 2, 1)).astype(BF16)
    maskT = np.ascontiguousarray(mask.transpose(0, 2, 1))  # (B, N, M)
    # packed tiles: [B, NMC, NT, 128, 512] int16; per-nt "one" constant
    one_nt = np.array(
        [SCHR_B if (nt % 16) in DVE16 else MB_ONE for nt in range(NT)],
        dtype=np.int16,
    )
    mt = maskT.reshape(B, NT, 128, NMC, 512)
    mb = np.where(mt, one_nt[None, :, None, None, None], np.int16(MB_ZERO))
    # (B, NMC, NG, 128, 4*512): groups of 4 nt-tiles per DMA
    mb = mb.reshape(B, NT // 4, 4, 128, NMC, 512)
    mb = np.ascontiguousarray(mb.transpose(0, 4, 1, 3, 2, 5)).reshape(
        B, NMC, NT // 4, 128, 4 * 512
    )

    nc = _get_nc((M, N, DQmat, Hc, DH))

    in_maps = []
    for core in range(8):
        b, hh = divmod(core, 2)
        hs = slice(hh * Hc * DH, (hh + 1) * Hc * DH)
        in_maps.append(
            {
                "leftT": leftT[b],
                "rightT": rightT[b],
                "maskb": mb[b],
                "wq": np.ascontiguousarray(Wqs[:, hs]),
                "wk": np.ascontiguousarray(Wk[:, hs]),
                "wv": np.ascontiguousarray(Wv[:, hs]),
                "wout": np.ascontiguousarray(WoutB[hs, :]),
            }
        )

    tmpdir = None
    if TRACE:
        import shutil

        shutil.rmtree("/tmp/attn_trace", ignore_errors=True)
        tmpdir = "/tmp/attn_trace"
    res = run_bass_kernel_spmd(nc, in_maps, list(range(8)), trace=TRACE, tmpdir=tmpdir)
    LAST_RESULTS = res

    out = np.zeros((B, M, DQmat), np.float32)
    for core in range(8):
        out[core // 2] += res.results[core]["out_p"]
    out += bout[None, None, :]
    return out


# revision 41
# speedup vs baseline: 1.0149x; 1.0149x over previous
"""Trainium2 Bass kernel for nn_Attention_62130996904205.

Full computation (reference):
    q = left @ Wq;  k,v = split(right @ Wkv)
    per head: S = scale * q k^T; S = where(mask, S, -1e7)
    out = (softmax(S) @ v) rearranged @ Wout + bout

Sharding: 8 cores = (batch b in 0..3) x (head-half in 0..1).  Host sums
the two head-half partials per batch and adds bout.

On-chip layout ("S^T scheme"): kv token index n stays on the partition
axis.  Per 128-token tile nt, BOTH heads of the pair live in one PSUM
tile s_nt[128, 2, 512] -- the two 64-contraction score matmuls (row
tiles (0,0) and (64,0)) share one buffer-release event so they issue
back-to-back and stream concurrently through the PE array.

exp+mask per nt uses one int16 mask tile (broadcast over the head dim):
  - Schraudolph nts: DVE fused  pm_bits = int16(A*s + mb),
        mb = mask ? 16249 : 5120   (A = 128/ln2 folded into Wq)
    B=16249 keeps the fast-exp mean-calibrated (+-3% sawtooth, no
    systematic bias vs the true-exp nts).  Masked entries ~1e-26.
  - true-exp nts: scalar-engine Exp then a DVE mask multiply with
        mb = mask ? 16256 : 5120   bitcast to bf16 = {1.0, 6.5e-27}.
O^T accumulates via [v | 1]-augmented matmuls, so softmax denominators
ride along free.
"""

import numpy as np
import ml_dtypes

import concourse.bass as bass
import concourse.mybir as mybir
import concourse.tile as tile
from concourse import bacc
from concourse.bass_utils import run_bass_kernel_spmd

BF16 = ml_dtypes.bfloat16

SCHR_A = 128.0 / np.log(2.0)          # 184.6627
SCHR_B = 16249                        # calibrated fast-exp bias
MB_ONE = 16256                        # bf16 bits of 1.0
MB_ZERO = 5120                        # positive-tiny for both paths

# nts (mod 16) handled by the DVE Schraudolph path; rest use ACT Exp.
DVE16 = frozenset({0, 2, 5, 7, 11, 14, 15})

TRACE = False
LAST_RESULTS = None


def build_core(M=1024, N=4096, DQ=512, H=4, DH=64):
    dt = mybir.dt
    f32, bf16, i16 = dt.float32, dt.bfloat16, dt.int16
    D = H * DH
    KT = DQ // 128
    NT = N // 128
    MCH = min(512, M)
    NMC = M // MCH
    SW = 2 * MCH
    DA = DH + 1
    KT2 = D // 128
    VW = H * DH

    assert M % MCH == 0 and N % 256 == 0 and DQ % 128 == 0 and D % 128 == 0

    nc = bacc.Bacc("TRN2", target_bir_lowering=False, debug=False)

    leftT = nc.dram_tensor("leftT", [DQ, M], bf16, kind="ExternalInput")
    rightT = nc.dram_tensor("rightT", [DQ, N], bf16, kind="ExternalInput")
    NG = NT // 4
    maskb = nc.dram_tensor("maskb", [NMC, NG, 128, 4 * MCH], i16, kind="ExternalInput")
    wq = nc.dram_tensor("wq", [DQ, D], bf16, kind="ExternalInput")
    wk = nc.dram_tensor("wk", [DQ, D], bf16, kind="ExternalInput")
    wv = nc.dram_tensor("wv", [DQ, D], bf16, kind="ExternalInput")
    wout = nc.dram_tensor("wout", [D, DQ], bf16, kind="ExternalInput")
    out_p = nc.dram_tensor("out_p", [M, DQ], f32, kind="ExternalOutput")

    EXP = mybir.ActivationFunctionType.Exp
    MUL = mybir.AluOpType.mult
    ADD = mybir.AluOpType.add

    with tile.TileContext(nc) as tc:
        with (
            tc.tile_pool(name="sing", bufs=1) as sing,
            tc.tile_pool(name="spool", bufs=3, space="PSUM") as spool,
            tc.tile_pool(name="opool", bufs=1, space="PSUM") as opool,
            tc.tile_pool(name="mpool", bufs=11) as mpool,
            tc.tile_pool(name="ppool", bufs=11) as ppool,
            tc.tile_pool(name="smallp", bufs=2) as smallp,
            tc.tile_pool(name="outp", bufs=5) as outp,
        ):
            # ---- bulk loads, all on the sync queue in dependency order ----
            wq_sb = sing.tile([128, KT, D], bf16, tag="wq")
            nc.sync.dma_start(out=wq_sb, in_=wq.rearrange("(kt p) d -> p kt d", p=128))
            leftT_sb = []
            for kt in range(KT):
                t = sing.tile([128, M], bf16, tag=f"leftT{kt}", name=f"leftT{kt}")
                nc.sync.dma_start(out=t, in_=leftT[kt * 128 : (kt + 1) * 128, :])
                leftT_sb.append(t)
            wk_sb = sing.tile([128, KT, D], bf16, tag="wk")
            nc.sync.dma_start(out=wk_sb, in_=wk.rearrange("(kt p) d -> p kt d", p=128))
            RCH = min(N, 1024)
            _EARLY_MASKS = True
            rightT_sb = [
                sing.tile([128, N], bf16, tag=f"rightT{kt}", name=f"rightT{kt}")
                for kt in range(KT)
            ]
            _early_msk_loads = []  # filled right below once load_msk_group exists
            wv_sb = sing.tile([128, KT, D], bf16, tag="wv")
            nc.sync.dma_start(out=wv_sb, in_=wv.rearrange("(kt p) d -> p kt d", p=128))

            qT2 = [sing.tile([128, M], bf16, tag=f"qT{h}", name=f"qT{h}") for h in range(H // 2)]
            kT2 = [sing.tile([128, N], bf16, tag=f"kT{h}", name=f"kT{h}") for h in range(H // 2)]
            u_sb = [sing.tile([128, M], bf16, tag=f"u{p}", name=f"u{p}") for p in range(KT2)]
            v_aug = sing.tile([128, NT, H, DA], bf16, tag="vaug")
            nc.vector.memset(v_aug[:, :, :, DH : DH + 1], 1.0)

            # masks: one [128, 4, MCH] int16 tile per (mc, 4-nt group), on the
            # sync queue (batched: 16 DMA issues total, not 64)
            msks = {}

            def load_msk_group(mc, g):
                mg = mpool.tile(
                    [128, 4, 1, MCH], i16, tag="msk", name=f"msk{mc}_{g}"
                )
                nc.sync.dma_start(out=mg, in_=maskb[mc, g])
                for j in range(4):
                    msks[(mc, 4 * g + j)] = (mg, j)

            # interleave remaining bulk with mc0 mask groups in deadline order
            for kt in range(KT):   # first column chunk: needed by k_chunk(0,0)
                nc.sync.dma_start(
                    out=rightT_sb[kt][:, 0:RCH],
                    in_=rightT[kt * 128 : (kt + 1) * 128, 0:RCH],
                )
            load_msk_group(0, 0)
            load_msk_group(0, 1)
            for c in range(1, N // RCH):
                for kt in range(KT):
                    nc.sync.dma_start(
                        out=rightT_sb[kt][:, c * RCH : (c + 1) * RCH],
                        in_=rightT[
                            kt * 128 : (kt + 1) * 128, c * RCH : (c + 1) * RCH
                        ],
                    )
                load_msk_group(0, 1 + c)
            for g in range(N // RCH + 1, NG):
                load_msk_group(0, g)
            wout_sb = sing.tile([128, KT2, DQ], bf16, tag="wout")
            nc.sync.dma_start(
                out=wout_sb, in_=wout.rearrange("(kt p) d -> p kt d", p=128)
            )

            # ---- q projection ----
            for t2 in range(H // 2):
                ps = spool.tile([128, 2, MCH], f32, tag="s")
                for mh in range(M // MCH):
                    for kt in range(KT):
                        nc.tensor.matmul(
                            ps[:, mh, :],
                            lhsT=wq_sb[:, kt, t2 * 128 : (t2 + 1) * 128],
                            rhs=leftT_sb[kt][:, mh * MCH : (mh + 1) * MCH],
                            start=(kt == 0),
                            stop=(kt == KT - 1),
                        )
                nc.scalar.copy(out=qT2[t2][:, :], in_=ps[:, :, :])

            CW = min(SW, N)
            NKC = N // CW

            def k_chunk(t2, cp):
                ps = spool.tile([128, 2, MCH], f32, tag="s", name="kps")
                for half in range(CW // MCH):
                    for kt in range(KT):
                        nc.tensor.matmul(
                            ps[:, half, :],
                            lhsT=wk_sb[:, kt, t2 * 128 : (t2 + 1) * 128],
                            rhs=rightT_sb[kt][
                                :, cp * CW + half * MCH : cp * CW + (half + 1) * MCH
                            ],
                            start=(kt == 0),
                            stop=(kt == KT - 1),
                        )
                nc.scalar.copy(
                    out=kT2[t2][:, cp * CW : (cp + 1) * CW], in_=ps[:, :, :]
                )

            def v_nt(nt):
                ps = spool.tile([128, 2, MCH], f32, tag="s", name="vps")
                for kt in range(KT):
                    nc.tensor.matmul(
                        ps[:, 0, 0:VW],
                        lhsT=rightT_sb[kt][:, nt * 128 : (nt + 1) * 128],
                        rhs=wv_sb[:, kt, :],
                        start=(kt == 0),
                        stop=(kt == KT - 1),
                    )
                nc.scalar.copy(out=v_aug[:, nt, :, 0:DH], in_=ps[:, 0, 0:VW])

            def outproj_mt(mt):
                ps = spool.tile([128, 2, MCH], f32, tag="s", name="ops")
                for p2 in range(KT2):
                    nc.tensor.matmul(
                        ps[:, 0, 0:DQ],
                        lhsT=u_sb[p2][:, mt * 128 : (mt + 1) * 128],
                        rhs=wout_sb[:, p2, :],
                        start=(p2 == 0),
                        stop=(p2 == KT2 - 1),
                    )
                ob = outp.tile([128, DQ], f32, tag="ob")
                nc.scalar.copy(out=ob, in_=ps[:, 0, 0:DQ])
                nc.sync.dma_start(out=out_p[mt * 128 : (mt + 1) * 128, :], in_=ob)

            # ---- upfront projection work (overlaps the initial DMA wave) ----
            UPFRONT_V = min(NT, 6)
            k_chunk(0, 0)
            for nt in range(UPFRONT_V):
                v_nt(nt)

            # ---- per-phase filler: (deadline_nt_slot, fn) sorted ----
            def phase_filler(mc, hp):
                items = []
                if mc == 0 and hp == 0:
                    for cp in range(1, NKC):
                        items.append((max(0, 8 * cp - 6), lambda cp=cp: k_chunk(0, cp)))
                    for nt in range(UPFRONT_V, NT):
                        items.append((max(0, nt - 5), lambda nt=nt: v_nt(nt)))
                    if H > 2:
                        # k(1,0) must be emitted before phase (0,1) reads it
                        items.append((NT - 1, lambda: k_chunk(1, 0)))
                elif mc == 0 and hp == 1 and H > 2:
                    for cp in range(1, NKC):
                        items.append((max(0, 8 * cp - 6), lambda cp=cp: k_chunk(1, cp)))
                elif mc == 1 and hp == 0:
                    for j in range(MCH // 128):
                        items.append((4 + 7 * j, lambda j=j: outproj_mt(j)))
                items.sort(key=lambda x: x[0])
                return items

            DEPTH = 5
            for mc in range(NMC):
                for hp in range(H // 2):
                    filler = phase_filler(mc, hp)
                    o_ps = [
                        opool.tile([DA, MCH], f32, tag=f"o{i}", name=f"o{i}")
                        for i in range(2)
                    ]
                    oq = []
                    started = [False, False]

                    def make_flush(o_ps, oq, started, hp):
                        def flush_one():
                            pm, nt_ = oq.pop(0)
                            for i in range(2):
                                nc.tensor.matmul(
                                    o_ps[i],
                                    lhsT=v_aug[:, nt_, 2 * hp + i, :],
                                    rhs=pm[:, i, :],
                                    start=(not started[i]),
                                    stop=(nt_ == NT - 1),
                                )
                                started[i] = True
                        return flush_one

                    flush_one = make_flush(o_ps, oq, started, hp)

                    for nt in range(NT):
                        mg, mj = msks[(mc, nt)]
                        # rolling prefetch of next-mc masks during hp=1
                        if hp == 1 and mc + 1 < NMC and nt % 4 == 0:
                            load_msk_group(mc + 1, nt // 4)
                        s_nt = spool.tile([128, 2, MCH], f32, tag="s", name=f"s{nt}")
                        for i in range(2):
                            lo = 64 * i
                            nc.tensor.matmul(
                                s_nt[:, i, :],
                                lhsT=kT2[hp][lo : lo + 64, nt * 128 : (nt + 1) * 128],
                                rhs=qT2[hp][lo : lo + 64, mc * MCH : (mc + 1) * MCH],
                                start=True,
                                stop=True,
                                tile_position=(lo, 0),
                            )
                        pm = ppool.tile([128, 2, MCH], bf16, tag="p")
                        if nt % 16 in DVE16:
                            nc.vector.scalar_tensor_tensor(
                                out=pm.bitcast(i16),
                                in0=s_nt[:, :, :],
                                scalar=1.0,
                                in1=mg[:, mj].to_broadcast((128, 2, MCH)),
                                op0=MUL,
                                op1=ADD,
                            )
                        else:
                            p_sb = ppool.tile([128, 2, MCH], bf16, tag="p")
                            nc.scalar.activation(
                                p_sb, s_nt, EXP, scale=float(1.0 / SCHR_A)
                            )
                            nc.vector.tensor_mul(
                                pm,
                                p_sb,
                                mg.bitcast(bf16)[:, mj].to_broadcast(
                                    (128, 2, MCH)
                                ),
                            )
                        oq.append((pm, nt))
                        if len(oq) > (DEPTH if nt < NT - 4 else 2):
                            flush_one()
                        while filler and filler[0][0] <= nt:
                            filler.pop(0)[1]()
                    while filler:
                        filler.pop(0)[1]()
                    while oq:
                        flush_one()
                    for i in range(2):
                        h = 2 * hp + i
                        rdc = smallp.tile([1, MCH], f32, tag="rdc", name=f"rdc{i}")
                        nc.scalar.copy(out=rdc, in_=o_ps[i][DH : DH + 1, :])
                        rd = smallp.tile([1, MCH], f32, tag="rd", name=f"rd{i}")
                        nc.vector.reciprocal_approx_fast(out=rd, in_=rdc)
                        bd = smallp.tile([64, MCH], f32, tag="bd", name=f"bd{i}")
                        nc.gpsimd.partition_broadcast(bd, rd)
                        nc.vector.tensor_mul(
                            u_sb[h // 2][
                                (h % 2) * 64 : (h % 2) * 64 + 64,
                                mc * MCH : (mc + 1) * MCH,
                            ],
                            o_ps[i][0:DH, :],
                            bd,
                        )
            # tail: last mc's output projections
            for mt in range((NMC - 1) * MCH // 128, NMC * MCH // 128):
                outproj_mt(mt)

    nc.finalize()
    return nc


_NC_CACHE = {}


def _get_nc(key=(1024, 4096, 512, 4, 64)):
    if key not in _NC_CACHE:
        _NC_CACHE[key] = build_core(*key)
    return _NC_CACHE[key]


def kernel(left, right, mask, Wq, Wkv, Wout, bout):
    """Full-input entry point: shards across 8 neuron cores, returns the
    full (B, M, DQ) output."""
    global LAST_RESULTS
    B, M, DQmat = left.shape
    _, N, DC = right.shape
    H, DH = 8, 64
    D = H * DH
    Hc = H // 2
    scale = DH ** -0.5
    NMC = M // 512
    NT = N // 128

    left = np.asarray(left, dtype=np.float32)
    right = np.asarray(right, dtype=np.float32)
    Wq = np.asarray(Wq, dtype=np.float32)
    Wkv = np.asarray(Wkv, dtype=np.float32)
    Wout = np.asarray(Wout, dtype=np.float32)
    bout = np.asarray(bout, dtype=np.float32)

    # 1/sqrt(DH) and the Schraudolph log2-scale are folded into Wq.
    # Wk,Wv are scaled x16 for fp8 e4m3 resolution; compensated by Wq/16
    # and Wout/16 respectively (the v scaling cancels in the softmax
    # denominator only for the value rows, so Wout absorbs it).
    Wqs = (Wq * (scale * SCHR_A)).astype(BF16)
    Wk = Wkv[:, :D].astype(BF16)
    Wv = Wkv[:, D:].astype(BF16)
    WoutB = Wout.astype(BF16)

    leftT = np.ascontiguousarray(left.transpose(0, 2, 1)).astype(BF16)
    rightT = np.ascontiguousarray(right.transpose(0, 2, 1)).astype(BF16)
    maskT = np.ascontiguousarray(mask.transpose(0, 2, 1))  # (B, N, M)
    # packed tiles: [B, NMC, NT, 128, 512] int16; per-nt "one" constant
    one_nt = np.array(
        [SCHR_B if (nt % 16) in DVE16 else MB_ONE for nt in range(NT)],
        dtype=np.int16,
    )
    mt = maskT.reshape(B, NT, 128, NMC, 512)
    mb = np.where(mt, one_nt[None, :, None, None, None], np.int16(MB_ZERO))
    # (B, NMC, NG, 128, 4*512): groups of 4 nt-tiles per DMA
    mb = mb.reshape(B, NT // 4, 4, 128, NMC, 512)
    mb = np.ascontiguousarray(mb.transpose(0, 4, 1, 3, 2, 5)).reshape(
        B, NMC, NT // 4, 128, 4 * 512
    )

    nc = _get_nc((M, N, DQmat, Hc, DH))

    in_maps = []
    for core in range(8):
        b, hh = divmod(core, 2)
        hs = slice(hh * Hc * DH, (hh + 1) * Hc * DH)
        in_maps.append(
            {
                "leftT": leftT[b],
                "rightT": rightT[b],
                "maskb": mb[b],
                "wq": np.ascontiguousarray(Wqs[:, hs]),
                "wk": np.ascontiguousarray(Wk[:, hs]),
                "wv": np.ascontiguousarray(Wv[:, hs]),
                "wout": np.ascontiguousarray(WoutB[hs, :]),
            }
        )

    tmpdir = None
    if TRACE:
        import shutil

        shutil.rmtree("/tmp/attn_trace", ignore_errors=True)
        tmpdir = "/tmp/attn_trace"
    res = run_bass_kernel_spmd(nc, in_maps, list(range(8)), trace=TRACE, tmpdir=tmpdir)
    LAST_RESULTS = res

    out = np.zeros((B, M, DQmat), np.float32)
    for core in range(8):
        out[core // 2] += res.results[core]["out_p"]
    out += bout[None, None, :]
    return out
